# revision 18
# baseline (speedup 1.0000x reference)
"""Multi-head causal self-attention on 8 Trainium2 NeuronCores (Bass/Tile).

Problem: x[2,2048,1024], 16 heads, d_k=64, causal softmax, out-proj + bias.

Sharding (tensor-parallel over heads + data-parallel over batch):
  core c in 0..7: batch b = c//4, heads 4*(c%4) .. 4*(c%4)+3.
  Each core: q/k/v projections in f32r (inputs pre-rounded on host, DMA'd
  straight into f32r tiles -- no on-device conversion copies), per-head
  causal attention in the transposed score layout S^T[s,t] with bf16
  activations (scores matmul bf16, exp on ACT -> bf16 P^T, attn@V bf16 with
  a ones column appended to V for the softmax denominators), causal mask
  applied via a bf16 identity/mask matmul accumulated into the score PSUM,
  denominators inverted on DVE + broadcast across partitions on GpSimd,
  per-head outputs packed into head-pair tiles [128, T] so the output
  projection runs K=128 matmuls, bias pre-broadcast once and fused into the
  PSUM->SBUF copy, then ReduceScatter(add) over each 4-core group which
  also hands every core a disjoint 512-row slice of y.

Per-rep input loads are software-pipelined: the body computes with tiles
loaded by the previous iteration and issues next iteration's DMAs at the
end, so loads overlap attention compute.
"""
import sys

sys.path.insert(0, "/opt/trn_rl_repo")

import numpy as np
import ml_dtypes
import concourse.bass as bass
import concourse.mybir as mybir
from concourse.bass_utils import run_bass_kernel_spmd
from concourse.tile import TileContext

FP32 = mybir.dt.float32
F32R = mybir.dt.float32r
BF16 = mybir.dt.bfloat16

B, T, C = 2, 2048, 1024
H, DK = 16, 64
NCORES = 8
HPC = 4            # heads per core
TB = T // 128      # 16 t-blocks
CB = C // 128      # 8 channel blocks
NCHUNK = T // 512  # 4 512-col output chunks
GROUPS = [[0, 1, 2, 3], [4, 5, 6, 7]]

_CACHE = {}


def _split_excess_waits(nc):
    """This walrus build encodes at most ONE sync wait per instruction.
    Hoist extras onto same-engine nops placed just before."""
    ctr = 0
    for f in nc.m.functions:
        for bb in f.blocks:
            new_insts = []
            changed = False
            for inst in bb.instructions:
                si = inst.sync_info
                if si is not None and si.on_wait and len(si.on_wait) > 1:
                    waits = list(si.on_wait)
                    for w in waits[:-1]:
                        ctr += 1
                        nop = mybir.InstNoOp(
                            name=f"I-waitsplit-{ctr}", ins=[], outs=[]
                        )
                        nop.engine = inst.engine
                        nop.sync_info = mybir.SyncInfo(on_wait=[w], on_update=[])
                        new_insts.append(nop)
                        changed = True
                    inst.sync_info = mybir.SyncInfo(
                        on_wait=[waits[-1]],
                        on_update=list(si.on_update) if si.on_update else [],
                    )
                new_insts.append(inst)
            if changed:
                bb.instructions = new_insts
    return ctr


def _rnd11(a):
    """Round fp32 array to the f32r grid (RNE to 11 mantissa bits) --
    bit-identical to the device's fp32->f32r tensor_copy."""
    a = np.ascontiguousarray(a, dtype=np.float32)
    b = a.view(np.uint32).astype(np.uint64)
    out = (((b + (((b >> 12) & 1) + 0x7FF)) >> 12) << 12).astype(np.uint32)
    return out.view(np.float32)


def build_program(n_reps=1, loop_always=False, parts="ABCL", with_collective=True,
                  unroll=False):
    """Emit the SPMD Bass program (same NEFF on all 8 cores)."""
    nc = bass.Bass("TRN2", target_bir_lowering=False, debug=False,
                   num_devices=NCORES)

    xt = nc.declare_dram_parameter("xt", [C, T], BF16, isOutput=False)
    wq = nc.declare_dram_parameter("wq", [128, CB * HPC * DK], BF16, isOutput=False)
    wk = nc.declare_dram_parameter("wk", [128, CB * HPC * DK], BF16, isOutput=False)
    wv = nc.declare_dram_parameter("wv", [128, CB * HPC * DK], BF16, isOutput=False)
    wot = nc.declare_dram_parameter("wot", [HPC * DK, C], BF16, isOutput=False)
    bo4 = nc.declare_dram_parameter("bo4", [1, C], F32R, isOutput=False)
    maskb = nc.declare_dram_parameter("maskb", [128, 128], BF16, isOutput=False)
    identb = nc.declare_dram_parameter("identb", [128, 128], BF16, isOutput=False)
    y = nc.declare_dram_parameter("y", [T // 4, C], FP32, isOutput=True)

    yp = nc.dram_tensor("yp", [T, C], BF16)       # partial y, pre-reduce
    ys = nc.dram_tensor("ys", [T // 4, C], BF16)  # my scattered slice

    with TileContext(nc) as tc:
        with (
            tc.tile_pool(name="const", bufs=1) as pc,
            tc.tile_pool(name="xtp", bufs=1) as px,
            tc.tile_pool(name="wp", bufs=1) as pw,
            tc.tile_pool(name="qk", bufs=1) as pqk,
            tc.tile_pool(name="v5p", bufs=1) as pv5,
            tc.tile_pool(name="outp", bufs=1) as pout,
        ):
            # ---- constants (once) ----
            maskt = pc.tile([128, 128], BF16)
            nc.sync.dma_start(out=maskt[:], in_=maskb[:])
            identt = pc.tile([128, 128], BF16)
            nc.gpsimd.dma_start(out=identt[:], in_=identb[:])
            bot = pc.tile([1, C], F32R)
            nc.sync.dma_start(out=bot[:], in_=bo4[:])
            ones1 = pc.tile([65, 128], F32R)
            nc.vector.memset(ones1[:].bitcast(FP32), 1.0)
            BBt = pc.tile([128, C], FP32)
            with tc.tile_pool(name="psBB", bufs=2, space="PSUM") as pBB:
                for dc in range(2):
                    psb = pBB.tile([128, 512], FP32, tag="psbb", name="psbb")
                    nc.tensor.matmul(psb[:], ones1[0:1, :],
                                     bot[:, dc * 512:(dc + 1) * 512],
                                     start=True, stop=True)
                    nc.vector.tensor_copy(BBt[:, dc * 512:(dc + 1) * 512],
                                          psb[:])

            # ---- persistent activation tiles ----
            XT = [px.tile([128, T], BF16, tag=f"xt{cb}", name=f"xt{cb}")
                  for cb in range(CB)]
            WQa = pw.tile([128, CB * HPC * DK], BF16, tag="wqa", name="wqa")
            WKa = pw.tile([128, CB * HPC * DK], BF16, tag="wka", name="wka")
            WVa = pw.tile([128, CB * HPC * DK], BF16, tag="wva", name="wva")
            WOT2 = [pw.tile([128, C], BF16, tag=f"wot{p}", name=f"wot{p}") for p in range(2)]
            QT = [pqk.tile([128, T], BF16, tag=f"qt{p}", name=f"qt{p}") for p in range(2)]
            KT = [pqk.tile([128, T], BF16, tag=f"kt{p}", name=f"kt{p}") for p in range(2)]
            V5 = [pv5.tile([128, HPC * 65], BF16, tag=f"v5_{tt}", name=f"v5_{tt}")
                  for tt in range(TB)]
            OUTT2 = [pout.tile([128, T], BF16, tag=f"out{p}", name=f"out{p}")
                     for p in range(2)]

            # V5 ones columns (constant across reps)
            for tt in range(TB):
                for h in range(HPC):
                    nc.vector.memset(V5[tt][:, 65 * h + 64:65 * h + 65], 1.0)

            consts = (maskt, identt, BBt, ones1)
            # prologue: rep-0 inputs
            _emit_loads_x(nc, XT, xt)
            _emit_loads_w(nc, WQa, WKa, WVa, WOT2, wq, wk, wv, wot)

            if n_reps == 1 and not loop_always:
                _emit_body(nc, tc, consts, XT, WQa, WKa, WVa, WOT2, QT, KT, V5,
                           OUTT2, xt, wq, wk, wv, wot, yp, parts)
            elif unroll:
                for _r in range(n_reps):
                    _emit_body(nc, tc, consts, XT, WQa, WKa, WVa, WOT2, QT, KT,
                               V5, OUTT2, xt, wq, wk, wv, wot, yp, parts)
            else:
                with tc.For_i(0, n_reps, 1) as _i:
                    _emit_body(nc, tc, consts, XT, WQa, WKa, WVa, WOT2, QT, KT,
                               V5, OUTT2, xt, wq, wk, wv, wot, yp, parts)
            if with_collective:
                _emit_ccout(nc, tc, consts, yp, ys, y)

    _split_excess_waits(nc)
    return nc


def _emit_loads_x(nc, XT, xt):
    engs = (nc.sync, nc.scalar)
    for cb in range(CB):
        engs[cb % 2].dma_start(out=XT[cb][:], in_=xt[cb * 128:(cb + 1) * 128, :])


def _emit_loads_w(nc, WQa, WKa, WVa, WOT2, wq, wk, wv, wot):
    """One batched DMA per weight tensor: dram [1024, 256] -> [128, 8, 256]."""
    engs = (nc.sync, nc.scalar)
    for i, (dst, src) in enumerate(((WQa, wq), (WKa, wk), (WVa, wv))):
        engs[i % 2].dma_start(out=dst[:], in_=src[:])
    for p in range(2):
        engs[p % 2].dma_start(
            out=WOT2[p][:], in_=wot[p * 128:(p + 1) * 128, :])


def _emit_qk_pair(nc, pA, XT, WQa, WKa, QT, KT, p):
    """Projections for head pair p: QT[p], KT[p] [128, T] bf16.
    QT[p][(h%2)*64 + d, t] = q_{2p+h%2}[t, d]."""
    for Wa, DST in ((WQa, QT), (WKa, KT)):
        for tch in range(NCHUNK):
            ps = pA.tile([128, 512], FP32, tag="ps", name="ps")
            for cb in range(CB):
                nc.tensor.matmul(
                    ps[:],
                    Wa[:, cb * 256 + p * 128:cb * 256 + (p + 1) * 128],
                    XT[cb][:, tch * 512:(tch + 1) * 512],
                    start=(cb == 0), stop=(cb == CB - 1))
            nc.vector.tensor_copy(
                DST[p][:, tch * 512:(tch + 1) * 512], ps[:])


def _emit_v(nc, pA, XT, WVa, V5):
    """v projection -> V5 tiles [128, 4*65] bf16 (ones cols pre-set)."""
    for tt in range(TB):
        ps = pA.tile([128, 512], FP32, tag="ps", name="ps")
        for cb in range(CB):
            nc.tensor.matmul(
                ps[:, 0:HPC * DK],
                XT[cb][:, tt * 128:(tt + 1) * 128],
                WVa[:, cb * 256:(cb + 1) * 256],
                start=(cb == 0), stop=(cb == CB - 1))
        out5 = V5[tt][:].rearrange("p (h c) -> p h c", c=65)[:, :, 0:64]
        in5 = ps[:, 0:HPC * DK].rearrange("p (h c) -> p h c", c=64)
        nc.vector.tensor_copy(out5, in5)


def _close_chunk(nc, pO, pst, praw, pstg, ones1, oc, OUTT2, p, r0, c):
    """Divide chunk c rows 0..63 by the sums row (64), write OUTT2."""
    raw = praw.tile([64, 512], FP32, tag="raw", name="raw")
    nc.vector.tensor_copy(raw[:], oc[c][0:64, :])
    rect = pst.tile([65, 512], F32R, tag="rect", name="rect")
    with nc.allow_low_precision(reason="softmax denom recip"):
        nc.vector.reciprocal(rect[64:65, :], oc[c][64:65, :])
    # broadcast 1/l across partitions via PE (reuses chunk c's freed bank)
    psb = pO.tile([128, 512], FP32, tag=f"oc{c}", name="psb")
    nc.tensor.matmul(psb[0:64, :], ones1[64:65, 0:64], rect[64:65, :],
                     start=True, stop=True)
    if r0 == 0:
        with nc.allow_low_precision(reason="f32r out"):
            nc.vector.tensor_tensor(
                out=OUTT2[p][0:64, 512 * c:512 * (c + 1)],
                in0=raw[:], in1=psb[0:64, :],
                op=mybir.AluOpType.mult)
    else:
        # DVE write with shifted partition base (in: 0..63, out: 64..127)
        with nc.allow_low_precision(reason="f32r out"):
            nc.vector.tensor_tensor(
                out=OUTT2[p][64:128, 512 * c:512 * (c + 1)],
                in0=raw[:], in1=psb[0:64, :],
                op=mybir.AluOpType.mult)


def _emit_scores_exp(nc, pS, maskt, identt, kh, qh, PT, jj):
    """S^T pieces for s-block jj -> exp -> PT.  1024-wide exp pieces."""
    cs = 128 * jj
    W = T - cs
    col = 0
    while col < W:
        w = min(1024, W - col)
        ps = pS.tile([128, 1024], FP32, tag="pss", name="pss")
        sub = 0
        while sub < w:
            sw = min(512, w - sub)
            diag = col == 0 and sub == 0
            nc.tensor.matmul(
                ps[:, sub:sub + sw],
                kh[:, cs:cs + 128],
                qh[:, cs + col + sub:cs + col + sub + sw],
                start=True, stop=not diag, skip_group_check=True)
            if diag:
                # causal mask: ps[:, 0:128] += I.T @ mask (adds -1e9 above diag)
                nc.tensor.matmul(
                    ps[:, 0:128], identt[:], maskt[:],
                    start=False, stop=True, skip_group_check=True)
            sub += sw
        nc.scalar.activation(
            PT[:, col:col + w], ps[:, 0:w],
            mybir.ActivationFunctionType.Exp, scale=0.125)
        col += w


def _emit_attnv(nc, oc, V5, PT, h, jj):
    """attn@V contributions of s-block jj to all open output chunks."""
    cs = 128 * jj
    for c in range(jj // 4, NCHUNK):
        lo = max(512 * c, cs)
        nc.tensor.matmul(
            oc[c][0:65, lo - 512 * c:512],
            V5[jj][:, 65 * h:65 * h + 65],
            PT[:, lo - cs:512 * (c + 1) - cs],
            start=(jj == 0), stop=(jj == 4 * c + 3),
            skip_group_check=True)


def _emit_pair(nc, pS, pO, ppt, pptb, pst, praw, pstg, consts,
               QT, KT, V5, OUTT2, p):
    """Fused causal attention for head pair (2p, 2p+1).

    Both heads' score matmuls are emitted adjacently each s-block so the PE
    runs them concurrently in disjoint row groups (K=64 row-tiling).  Head
    2p's attn@V runs inline; head 2p+1's P^T tiles persist and its attn@V
    runs as a dense PE tail with no ACT dependency."""
    maskt, identt, _, ones1 = consts
    a, b = 2 * p, 2 * p + 1
    qa, ka = QT[p][0:64, :], KT[p][0:64, :]
    qb, kb = QT[p][64:128, :], KT[p][64:128, :]
    oca = [pO.tile([128, 512], FP32, tag=f"oc{c}", name=f"oc{c}")
           for c in range(NCHUNK)]
    PTB = []
    for jj in range(TB):
        W = T - 128 * jj
        PTA = ppt.tile([128, W], BF16, tag="pta", name="pta")
        PTb = pptb.tile([128, W], BF16, tag=f"ptb{jj}", name=f"ptb{jj}")
        PTB.append(PTb)
        _emit_scores_exp(nc, pS, maskt, identt, ka, qa, PTA, jj)
        _emit_scores_exp(nc, pS, maskt, identt, kb, qb, PTb, jj)
        _emit_attnv(nc, oca, V5, PTA, a, jj)
        for c in range(jj // 4, NCHUNK):
            if jj == 4 * c + 3:
                _close_chunk(nc, pO, pst, praw, pstg, ones1, oca, OUTT2, p, 0, c)
    # head b: dense attn@V tail (PT tiles already in SBUF)
    ocb = [pO.tile([128, 512], FP32, tag=f"oc{c}", name=f"oc{c}")
           for c in range(NCHUNK)]
    for jj in range(TB):
        _emit_attnv(nc, ocb, V5, PTB[jj], b, jj)
        for c in range(jj // 4, NCHUNK):
            if jj == 4 * c + 3:
                _close_chunk(nc, pO, pst, praw, pstg, ones1, ocb, OUTT2, p, 64, c)


def _emit_c(nc, pC, pys, OUTT2, WOT2, yp):
    """Output projection (K=128 per pair) + store to yp (bias in epilogue)."""
    for tblk in range(TB):
        ysb = pys.tile([128, 1024], BF16, tag="ysb", name="ysb")
        for dc in range(2):
            ps = pC.tile([128, 512], FP32, tag="psy", name="psy")
            for p in range(2):
                nc.tensor.matmul(
                    ps[:],
                    OUTT2[p][:, tblk * 128:(tblk + 1) * 128],
                    WOT2[p][:, dc * 512:(dc + 1) * 512],
                    start=(p == 0), stop=(p == 1))
            if dc == 0:
                nc.scalar.activation(
                    ysb[:, 0:512], ps[:],
                    mybir.ActivationFunctionType.Copy)
            else:
                nc.vector.tensor_copy(ysb[:, 512:1024], ps[:])
        engs = (nc.sync, nc.scalar)
        engs[tblk % 2].dma_start(
            out=yp[tblk * 128:(tblk + 1) * 128, :], in_=ysb[:])


def _emit_body(nc, tc, consts, XT, WQa, WKa, WVa, WOT2, QT, KT, V5, OUTT2,
               xt, wq, wk, wv, wot, yp, parts="ABC"):
    _, _, BBt, _ = consts
    if "A" in parts:
        with tc.tile_pool(name="psA", bufs=2, space="PSUM") as pA:
            _emit_qk_pair(nc, pA, XT, WQa, WKa, QT, KT, 0)
            _emit_qk_pair(nc, pA, XT, WQa, WKa, QT, KT, 1)
            _emit_v(nc, pA, XT, WVa, V5)
    # next iteration's x (XT free after the projections above; the DMA
    # overlaps this iteration's attention + output phases)
    if "L" in parts:
        _emit_loads_x(nc, XT, xt)
    if "B" in parts:
        with (
            tc.tile_pool(name="psS", bufs=2, space="PSUM") as pS,
            tc.tile_pool(name="psO", bufs=1, space="PSUM") as pO,
            tc.tile_pool(name="ptp", bufs=2) as ppt,
            tc.tile_pool(name="ptbp", bufs=1) as pptb,
            tc.tile_pool(name="rectp", bufs=2) as pst,
            tc.tile_pool(name="rawp", bufs=2) as praw,
            tc.tile_pool(name="stgp", bufs=2) as pstg,
        ):
            for p in range(2):
                _emit_pair(nc, pS, pO, ppt, pptb, pst, praw, pstg, consts,
                           QT, KT, V5, OUTT2, p)
    if "C" in parts:
        with (
            tc.tile_pool(name="psC", bufs=2, space="PSUM") as pC,
            tc.tile_pool(name="ysb", bufs=3) as pys,
        ):
            _emit_c(nc, pC, pys, OUTT2, WOT2, yp)
    # weights for next iteration (overlap nothing critical; small)
    if "L" in parts:
        _emit_loads_w(nc, WQa, WKa, WVa, WOT2, wq, wk, wv, wot)


def _emit_ccout(nc, tc, consts, yp, ys, y):
    BBt = consts[2]
    with tc.tile_pool(name="ccb", bufs=2) as pcc:
        nc.gpsimd.collective_compute(
            "ReduceScatter", mybir.AluOpType.add,
            ins=[yp[:]], outs=[ys[:]], replica_groups=GROUPS)
        # ys + bias -> external output (bounce through SBUF)
        for i in range(4):
            t = pcc.tile([128, C], BF16, tag="ybounce", name="ybounce")
            nc.sync.dma_start(out=t[:], in_=ys[i * 128:(i + 1) * 128, :])
            tb = pcc.tile([128, C], FP32, tag="ybias", name="ybias")
            nc.vector.tensor_tensor(out=tb[:], in0=t[:], in1=BBt[:],
                                    op=mybir.AluOpType.add)
            nc.sync.dma_start(out=y[i * 128:(i + 1) * 128, :], in_=tb[:])


def _make_in_maps(x, Wq, Wk, Wv, Wo, bo):
    mask = np.where(
        np.arange(128)[:, None] <= np.arange(128)[None, :], 0.0, -1e9
    ).astype(ml_dtypes.bfloat16)
    ident = np.eye(128, dtype=ml_dtypes.bfloat16)
    in_maps = []
    for c in range(NCORES):
        b, hh = c // 4, HPC * (c % 4)
        ch0 = hh * DK
        bf = ml_dtypes.bfloat16

        def swz(w):
            # [C, 256] -> [128, cb-major 256] so the tile DMA is contiguous
            w = np.concatenate([w[hh + i] for i in range(HPC)], axis=1)
            return np.ascontiguousarray(
                w.reshape(CB, 128, HPC * DK).transpose(1, 0, 2)
                .reshape(128, CB * HPC * DK)).astype(bf)

        in_maps.append({
            "xt": np.ascontiguousarray(x[b].T).astype(bf),
            "wq": swz(Wq),
            "wk": swz(Wk),
            "wv": swz(Wv),
            "wot": np.ascontiguousarray(Wo[:, ch0:ch0 + HPC * DK].T).astype(bf),
            "bo4": _rnd11(bo.reshape(1, C)),
            "maskb": mask,
            "identb": ident,
        })
    return in_maps


def kernel(x, Wq, Wk, Wv, Wo, bo):
    x = np.asarray(x, dtype=np.float32)
    Wq = np.asarray(Wq, dtype=np.float32)
    Wk = np.asarray(Wk, dtype=np.float32)
    Wv = np.asarray(Wv, dtype=np.float32)
    Wo = np.asarray(Wo, dtype=np.float32)
    bo = np.asarray(bo, dtype=np.float32)

    if "nc" not in _CACHE:
        _CACHE["nc"] = build_program()
    nc = _CACHE["nc"]

    in_maps = _make_in_maps(x, Wq, Wk, Wv, Wo, bo)
    res = run_bass_kernel_spmd(nc, in_maps, list(range(NCORES)))

    out = np.empty((B, T, C), dtype=np.float32)
    for c in range(NCORES):
        b, r = c // 4, c % 4
        out[b, r * 512:(r + 1) * 512, :] = res.results[c]["y"]
    return out


# revision 20
# speedup vs baseline: 1.0829x; 1.0829x over previous
"""Multi-head causal self-attention on 8 Trainium2 NeuronCores (Bass/Tile).

Problem: x[2,2048,1024], 16 heads, d_k=64, causal softmax, out-proj + bias.

Sharding (tensor-parallel over heads + data-parallel over batch):
  core c in 0..7: batch b = c//4, heads 4*(c%4) .. 4*(c%4)+3.
  Each core: q/k/v projections in f32r (inputs pre-rounded on host, DMA'd
  straight into f32r tiles -- no on-device conversion copies), per-head
  causal attention in the transposed score layout S^T[s,t] with bf16
  activations (scores matmul bf16, exp on ACT -> bf16 P^T, attn@V bf16 with
  a ones column appended to V for the softmax denominators), causal mask
  applied via a bf16 identity/mask matmul accumulated into the score PSUM,
  denominators inverted on DVE + broadcast across partitions on GpSimd,
  per-head outputs packed into head-pair tiles [128, T] so the output
  projection runs K=128 matmuls, bias pre-broadcast once and fused into the
  PSUM->SBUF copy, then ReduceScatter(add) over each 4-core group which
  also hands every core a disjoint 512-row slice of y.

Per-rep input loads are software-pipelined: the body computes with tiles
loaded by the previous iteration and issues next iteration's DMAs at the
end, so loads overlap attention compute.
"""
import sys

sys.path.insert(0, "/opt/trn_rl_repo")

import numpy as np
import ml_dtypes
import concourse.bass as bass
import concourse.mybir as mybir
from concourse.bass_utils import run_bass_kernel_spmd
from concourse.tile import TileContext

FP32 = mybir.dt.float32
F32R = mybir.dt.float32r
BF16 = mybir.dt.bfloat16

B, T, C = 2, 2048, 1024
H, DK = 16, 64
NCORES = 8
HPC = 4            # heads per core
TB = T // 128      # 16 t-blocks
CB = C // 128      # 8 channel blocks
NCHUNK = T // 512  # 4 512-col output chunks
EXPW = 512        # exp piece width
GROUPS = [[0, 1, 2, 3], [4, 5, 6, 7]]

_CACHE = {}


def _split_excess_waits(nc):
    """This walrus build encodes at most ONE sync wait per instruction.
    Hoist extras onto same-engine nops placed just before."""
    ctr = 0
    for f in nc.m.functions:
        for bb in f.blocks:
            new_insts = []
            changed = False
            for inst in bb.instructions:
                si = inst.sync_info
                if si is not None and si.on_wait and len(si.on_wait) > 1:
                    waits = list(si.on_wait)
                    for w in waits[:-1]:
                        ctr += 1
                        nop = mybir.InstNoOp(
                            name=f"I-waitsplit-{ctr}", ins=[], outs=[]
                        )
                        nop.engine = inst.engine
                        nop.sync_info = mybir.SyncInfo(on_wait=[w], on_update=[])
                        new_insts.append(nop)
                        changed = True
                    inst.sync_info = mybir.SyncInfo(
                        on_wait=[waits[-1]],
                        on_update=list(si.on_update) if si.on_update else [],
                    )
                new_insts.append(inst)
            if changed:
                bb.instructions = new_insts
    return ctr


def _rnd11(a):
    """Round fp32 array to the f32r grid (RNE to 11 mantissa bits) --
    bit-identical to the device's fp32->f32r tensor_copy."""
    a = np.ascontiguousarray(a, dtype=np.float32)
    b = a.view(np.uint32).astype(np.uint64)
    out = (((b + (((b >> 12) & 1) + 0x7FF)) >> 12) << 12).astype(np.uint32)
    return out.view(np.float32)


def build_program(n_reps=1, loop_always=False, parts="ABCL", with_collective=True,
                  unroll=False):
    """Emit the SPMD Bass program (same NEFF on all 8 cores)."""
    nc = bass.Bass("TRN2", target_bir_lowering=False, debug=False,
                   num_devices=NCORES)

    xt = nc.declare_dram_parameter("xt", [C, T], BF16, isOutput=False)
    wq = nc.declare_dram_parameter("wq", [128, CB * HPC * DK], BF16, isOutput=False)
    wk = nc.declare_dram_parameter("wk", [128, CB * HPC * DK], BF16, isOutput=False)
    wv = nc.declare_dram_parameter("wv", [128, CB * HPC * DK], BF16, isOutput=False)
    wot = nc.declare_dram_parameter("wot", [HPC * DK, C], BF16, isOutput=False)
    bo4 = nc.declare_dram_parameter("bo4", [1, C], F32R, isOutput=False)
    maskb = nc.declare_dram_parameter("maskb", [128, 128], BF16, isOutput=False)
    identb = nc.declare_dram_parameter("identb", [128, 128], BF16, isOutput=False)
    y = nc.declare_dram_parameter("y", [T // 4, C], FP32, isOutput=True)

    yp = nc.dram_tensor("yp", [T, C], BF16)       # partial y, pre-reduce
    ys = nc.dram_tensor("ys", [T // 4, C], BF16)  # my scattered slice

    with TileContext(nc) as tc:
        with (
            tc.tile_pool(name="const", bufs=1) as pc,
            tc.tile_pool(name="xtp", bufs=1) as px,
            tc.tile_pool(name="wp", bufs=1) as pw,
            tc.tile_pool(name="qk", bufs=1) as pqk,
            tc.tile_pool(name="v5p", bufs=1) as pv5,
            tc.tile_pool(name="outp", bufs=1) as pout,
        ):
            # ---- constants (once) ----
            maskt = pc.tile([128, 128], BF16)
            nc.sync.dma_start(out=maskt[:], in_=maskb[:])
            identt = pc.tile([128, 128], BF16)
            nc.gpsimd.dma_start(out=identt[:], in_=identb[:])
            bot = pc.tile([1, C], F32R)
            nc.sync.dma_start(out=bot[:], in_=bo4[:])
            ones1 = pc.tile([65, 128], F32R)
            nc.vector.memset(ones1[:].bitcast(FP32), 1.0)
            BBt = pc.tile([128, C], FP32)
            with tc.tile_pool(name="psBB", bufs=2, space="PSUM") as pBB:
                for dc in range(2):
                    psb = pBB.tile([128, 512], FP32, tag="psbb", name="psbb")
                    nc.tensor.matmul(psb[:], ones1[0:1, :],
                                     bot[:, dc * 512:(dc + 1) * 512],
                                     start=True, stop=True)
                    nc.vector.tensor_copy(BBt[:, dc * 512:(dc + 1) * 512],
                                          psb[:])

            # ---- persistent activation tiles ----
            XT = [px.tile([128, T], BF16, tag=f"xt{cb}", name=f"xt{cb}")
                  for cb in range(CB)]
            WQa = pw.tile([128, CB * HPC * DK], BF16, tag="wqa", name="wqa")
            WKa = pw.tile([128, CB * HPC * DK], BF16, tag="wka", name="wka")
            WVa = pw.tile([128, CB * HPC * DK], BF16, tag="wva", name="wva")
            WOT2 = [pw.tile([128, C], BF16, tag=f"wot{p}", name=f"wot{p}") for p in range(2)]
            QT = [pqk.tile([128, T], BF16, tag=f"qt{p}", name=f"qt{p}") for p in range(2)]
            KT = [pqk.tile([128, T], BF16, tag=f"kt{p}", name=f"kt{p}") for p in range(2)]
            V5 = [pv5.tile([128, HPC * 65], BF16, tag=f"v5_{tt}", name=f"v5_{tt}")
                  for tt in range(TB)]
            OUTT2 = [pout.tile([128, T], BF16, tag=f"out{p}", name=f"out{p}")
                     for p in range(2)]

            # V5 ones columns (constant across reps)
            for tt in range(TB):
                for h in range(HPC):
                    nc.vector.memset(V5[tt][:, 65 * h + 64:65 * h + 65], 1.0)

            consts = (maskt, identt, BBt, ones1)
            # prologue: rep-0 inputs
            _emit_loads_x(nc, XT, xt)
            _emit_loads_w(nc, WQa, WKa, WVa, WOT2, wq, wk, wv, wot)

            if n_reps == 1 and not loop_always:
                _emit_body(nc, tc, consts, XT, WQa, WKa, WVa, WOT2, QT, KT, V5,
                           OUTT2, xt, wq, wk, wv, wot, yp, parts)
            elif unroll:
                for _r in range(n_reps):
                    _emit_body(nc, tc, consts, XT, WQa, WKa, WVa, WOT2, QT, KT,
                               V5, OUTT2, xt, wq, wk, wv, wot, yp, parts)
            else:
                with tc.For_i(0, n_reps, 1) as _i:
                    _emit_body(nc, tc, consts, XT, WQa, WKa, WVa, WOT2, QT, KT,
                               V5, OUTT2, xt, wq, wk, wv, wot, yp, parts)
            if with_collective:
                _emit_ccout(nc, tc, consts, yp, ys, y)

    _split_excess_waits(nc)
    return nc


def _emit_loads_x(nc, XT, xt):
    engs = (nc.sync, nc.scalar)
    for cb in range(CB):
        engs[cb % 2].dma_start(out=XT[cb][:], in_=xt[cb * 128:(cb + 1) * 128, :])


def _emit_loads_w(nc, WQa, WKa, WVa, WOT2, wq, wk, wv, wot):
    """One batched DMA per weight tensor: dram [1024, 256] -> [128, 8, 256]."""
    engs = (nc.sync, nc.scalar)
    for i, (dst, src) in enumerate(((WQa, wq), (WKa, wk), (WVa, wv))):
        engs[i % 2].dma_start(out=dst[:], in_=src[:])
    for p in range(2):
        engs[p % 2].dma_start(
            out=WOT2[p][:], in_=wot[p * 128:(p + 1) * 128, :])


def _emit_qk_pair(nc, pA, XT, WQa, WKa, QT, KT, p):
    """Projections for head pair p: QT[p], KT[p] [128, T] bf16.
    QT[p][(h%2)*64 + d, t] = q_{2p+h%2}[t, d]."""
    for Wa, DST in ((WQa, QT), (WKa, KT)):
        for tch in range(NCHUNK):
            ps = pA.tile([128, 512], FP32, tag="ps", name="ps")
            for cb in range(CB):
                nc.tensor.matmul(
                    ps[:],
                    Wa[:, cb * 256 + p * 128:cb * 256 + (p + 1) * 128],
                    XT[cb][:, tch * 512:(tch + 1) * 512],
                    start=(cb == 0), stop=(cb == CB - 1))
            nc.vector.tensor_copy(
                DST[p][:, tch * 512:(tch + 1) * 512], ps[:])


def _emit_v(nc, pA, XT, WVa, V5):
    """v projection -> V5 tiles [128, 4*65] bf16 (ones cols pre-set)."""
    for tt in range(TB):
        ps = pA.tile([128, 512], FP32, tag="ps", name="ps")
        for cb in range(CB):
            nc.tensor.matmul(
                ps[:, 0:HPC * DK],
                XT[cb][:, tt * 128:(tt + 1) * 128],
                WVa[:, cb * 256:(cb + 1) * 256],
                start=(cb == 0), stop=(cb == CB - 1))
        out5 = V5[tt][:].rearrange("p (h c) -> p h c", c=65)[:, :, 0:64]
        in5 = ps[:, 0:HPC * DK].rearrange("p (h c) -> p h c", c=64)
        nc.vector.tensor_copy(out5, in5)


def _close_chunk(nc, pO, pst, praw, pstg, ones1, oc, OUTT2, p, r0, c):
    """Divide chunk c rows 0..63 by the sums row (64), write OUTT2."""
    raw = praw.tile([64, 512], FP32, tag="raw", name="raw")
    nc.vector.tensor_copy(raw[:], oc[c][0:64, :])
    rect = pst.tile([65, 512], F32R, tag="rect", name="rect")
    with nc.allow_low_precision(reason="softmax denom recip"):
        nc.vector.reciprocal(rect[64:65, :], oc[c][64:65, :])
    # broadcast 1/l across partitions via PE (reuses chunk c's freed bank)
    psb = pO.tile([128, 512], FP32, tag=f"oc{c}", name="psb")
    nc.tensor.matmul(psb[0:64, :], ones1[64:65, 0:64], rect[64:65, :],
                     start=True, stop=True)
    if r0 == 0:
        with nc.allow_low_precision(reason="f32r out"):
            nc.vector.tensor_tensor(
                out=OUTT2[p][0:64, 512 * c:512 * (c + 1)],
                in0=raw[:], in1=psb[0:64, :],
                op=mybir.AluOpType.mult)
    else:
        # DVE write with shifted partition base (in: 0..63, out: 64..127)
        with nc.allow_low_precision(reason="f32r out"):
            nc.vector.tensor_tensor(
                out=OUTT2[p][64:128, 512 * c:512 * (c + 1)],
                in0=raw[:], in1=psb[0:64, :],
                op=mybir.AluOpType.mult)


def _emit_scores_exp(nc, pS, maskt, identt, kh, qh, PT, jj):
    """S^T pieces for s-block jj -> exp -> PT.  1024-wide exp pieces."""
    cs = 128 * jj
    W = T - cs
    col = 0
    while col < W:
        w = min(EXPW, W - col)
        ps = pS.tile([128, EXPW], FP32, tag="pss", name="pss")
        sub = 0
        while sub < w:
            sw = min(512, w - sub)
            diag = col == 0 and sub == 0
            nc.tensor.matmul(
                ps[:, sub:sub + sw],
                kh[:, cs:cs + 128],
                qh[:, cs + col + sub:cs + col + sub + sw],
                start=True, stop=not diag, skip_group_check=True)
            if diag:
                # causal mask: ps[:, 0:128] += I.T @ mask (adds -1e9 above diag)
                nc.tensor.matmul(
                    ps[:, 0:128], identt[:], maskt[:],
                    start=False, stop=True, skip_group_check=True)
            sub += sw
        nc.scalar.activation(
            PT[:, col:col + w], ps[:, 0:w],
            mybir.ActivationFunctionType.Exp, scale=0.125)
        col += w


def _emit_attnv(nc, oc, V5, PT, h, jj):
    """attn@V contributions of s-block jj to all open output chunks."""
    cs = 128 * jj
    for c in range(jj // 4, NCHUNK):
        lo = max(512 * c, cs)
        nc.tensor.matmul(
            oc[c][0:65, lo - 512 * c:512],
            V5[jj][:, 65 * h:65 * h + 65],
            PT[:, lo - cs:512 * (c + 1) - cs],
            start=(jj == 0), stop=(jj == 4 * c + 3),
            skip_group_check=True)


def _emit_pair(nc, pS, pO, ppt, pptb, pst, praw, pstg, consts,
               QT, KT, V5, OUTT2, p):
    """Fused causal attention for head pair (2p, 2p+1).

    Both heads' score matmuls are emitted adjacently each s-block so the PE
    runs them concurrently in disjoint row groups (K=64 row-tiling).  Head
    2p's attn@V runs inline; head 2p+1's P^T tiles persist and its attn@V
    runs as a dense PE tail with no ACT dependency."""
    maskt, identt, _, ones1 = consts
    a, b = 2 * p, 2 * p + 1
    qa, ka = QT[p][0:64, :], KT[p][0:64, :]
    qb, kb = QT[p][64:128, :], KT[p][64:128, :]
    oca = [pO.tile([128, 512], FP32, tag=f"oc{c}", name=f"oc{c}")
           for c in range(NCHUNK)]
    PTB = []
    for jj in range(TB):
        W = T - 128 * jj
        PTA = ppt.tile([128, W], BF16, tag="pta", name="pta")
        PTb = pptb.tile([128, W], BF16, tag=f"ptb{jj}", name=f"ptb{jj}")
        PTB.append(PTb)
        _emit_scores_exp(nc, pS, maskt, identt, ka, qa, PTA, jj)
        _emit_scores_exp(nc, pS, maskt, identt, kb, qb, PTb, jj)
        _emit_attnv(nc, oca, V5, PTA, a, jj)
        for c in range(jj // 4, NCHUNK):
            if jj == 4 * c + 3:
                _close_chunk(nc, pO, pst, praw, pstg, ones1, oca, OUTT2, p, 0, c)
    # head b: dense attn@V tail (PT tiles already in SBUF)
    ocb = [pO.tile([128, 512], FP32, tag=f"oc{c}", name=f"oc{c}")
           for c in range(NCHUNK)]
    for jj in range(TB):
        _emit_attnv(nc, ocb, V5, PTB[jj], b, jj)
        for c in range(jj // 4, NCHUNK):
            if jj == 4 * c + 3:
                _close_chunk(nc, pO, pst, praw, pstg, ones1, ocb, OUTT2, p, 64, c)


def _emit_c(nc, pC, pys, OUTT2, WOT2, yp):
    """Output projection (K=128 per pair) + store to yp (bias in epilogue)."""
    for tblk in range(TB):
        ysb = pys.tile([128, 1024], BF16, tag="ysb", name="ysb")
        for dc in range(2):
            ps = pC.tile([128, 512], FP32, tag="psy", name="psy")
            for p in range(2):
                nc.tensor.matmul(
                    ps[:],
                    OUTT2[p][:, tblk * 128:(tblk + 1) * 128],
                    WOT2[p][:, dc * 512:(dc + 1) * 512],
                    start=(p == 0), stop=(p == 1))
            if dc == 0:
                nc.scalar.activation(
                    ysb[:, 0:512], ps[:],
                    mybir.ActivationFunctionType.Copy)
            else:
                nc.vector.tensor_copy(ysb[:, 512:1024], ps[:])
        engs = (nc.sync, nc.scalar)
        engs[tblk % 2].dma_start(
            out=yp[tblk * 128:(tblk + 1) * 128, :], in_=ysb[:])


def _emit_body(nc, tc, consts, XT, WQa, WKa, WVa, WOT2, QT, KT, V5, OUTT2,
               xt, wq, wk, wv, wot, yp, parts="ABC"):
    _, _, BBt, _ = consts
    if "A" in parts:
        with tc.tile_pool(name="psA", bufs=2, space="PSUM") as pA:
            _emit_qk_pair(nc, pA, XT, WQa, WKa, QT, KT, 0)
            _emit_qk_pair(nc, pA, XT, WQa, WKa, QT, KT, 1)
            _emit_v(nc, pA, XT, WVa, V5)
    # next iteration's x (XT free after the projections above; the DMA
    # overlaps this iteration's attention + output phases)
    if "L" in parts:
        _emit_loads_x(nc, XT, xt)
    if "B" in parts:
        with (
            tc.tile_pool(name="psS", bufs=4, space="PSUM") as pS,
            tc.tile_pool(name="psO", bufs=1, space="PSUM") as pO,
            tc.tile_pool(name="ptp", bufs=3) as ppt,
            tc.tile_pool(name="ptbp", bufs=1) as pptb,
            tc.tile_pool(name="rectp", bufs=2) as pst,
            tc.tile_pool(name="rawp", bufs=2) as praw,
            tc.tile_pool(name="stgp", bufs=2) as pstg,
        ):
            for p in range(2):
                _emit_pair(nc, pS, pO, ppt, pptb, pst, praw, pstg, consts,
                           QT, KT, V5, OUTT2, p)
    if "C" in parts:
        with (
            tc.tile_pool(name="psC", bufs=2, space="PSUM") as pC,
            tc.tile_pool(name="ysb", bufs=3) as pys,
        ):
            _emit_c(nc, pC, pys, OUTT2, WOT2, yp)
    # weights for next iteration (overlap nothing critical; small)
    if "L" in parts:
        _emit_loads_w(nc, WQa, WKa, WVa, WOT2, wq, wk, wv, wot)


def _emit_ccout(nc, tc, consts, yp, ys, y):
    BBt = consts[2]
    with tc.tile_pool(name="ccb", bufs=2) as pcc:
        nc.gpsimd.collective_compute(
            "ReduceScatter", mybir.AluOpType.add,
            ins=[yp[:]], outs=[ys[:]], replica_groups=GROUPS)
        # ys + bias -> external output (bounce through SBUF)
        for i in range(4):
            t = pcc.tile([128, C], BF16, tag="ybounce", name="ybounce")
            nc.sync.dma_start(out=t[:], in_=ys[i * 128:(i + 1) * 128, :])
            tb = pcc.tile([128, C], FP32, tag="ybias", name="ybias")
            nc.vector.tensor_tensor(out=tb[:], in0=t[:], in1=BBt[:],
                                    op=mybir.AluOpType.add)
            nc.sync.dma_start(out=y[i * 128:(i + 1) * 128, :], in_=tb[:])


def _make_in_maps(x, Wq, Wk, Wv, Wo, bo):
    mask = np.where(
        np.arange(128)[:, None] <= np.arange(128)[None, :], 0.0, -1e9
    ).astype(ml_dtypes.bfloat16)
    ident = np.eye(128, dtype=ml_dtypes.bfloat16)
    in_maps = []
    for c in range(NCORES):
        b, hh = c // 4, HPC * (c % 4)
        ch0 = hh * DK
        bf = ml_dtypes.bfloat16

        def swz(w):
            # [C, 256] -> [128, cb-major 256] so the tile DMA is contiguous
            w = np.concatenate([w[hh + i] for i in range(HPC)], axis=1)
            return np.ascontiguousarray(
                w.reshape(CB, 128, HPC * DK).transpose(1, 0, 2)
                .reshape(128, CB * HPC * DK)).astype(bf)

        in_maps.append({
            "xt": np.ascontiguousarray(x[b].T).astype(bf),
            "wq": swz(Wq),
            "wk": swz(Wk),
            "wv": swz(Wv),
            "wot": np.ascontiguousarray(Wo[:, ch0:ch0 + HPC * DK].T).astype(bf),
            "bo4": _rnd11(bo.reshape(1, C)),
            "maskb": mask,
            "identb": ident,
        })
    return in_maps


def kernel(x, Wq, Wk, Wv, Wo, bo):
    x = np.asarray(x, dtype=np.float32)
    Wq = np.asarray(Wq, dtype=np.float32)
    Wk = np.asarray(Wk, dtype=np.float32)
    Wv = np.asarray(Wv, dtype=np.float32)
    Wo = np.asarray(Wo, dtype=np.float32)
    bo = np.asarray(bo, dtype=np.float32)

    if "nc" not in _CACHE:
        _CACHE["nc"] = build_program()
    nc = _CACHE["nc"]

    in_maps = _make_in_maps(x, Wq, Wk, Wv, Wo, bo)
    res = run_bass_kernel_spmd(nc, in_maps, list(range(NCORES)))

    out = np.empty((B, T, C), dtype=np.float32)
    for c in range(NCORES):
        b, r = c // 4, c % 4
        out[b, r * 512:(r + 1) * 512, :] = res.results[c]["y"]
    return out
